# revision 1
# baseline (speedup 1.0000x reference)
"""BloomMaskDistillationLoss on Trainium2 — SPMD Bass kernel over 8 NeuronCores.

Math (EPS = 1e-12), for inputs full_emb f [B, D], query_mask m [B, D]:
  sim_full[i,j]   = <f_i, f_j>
  num[i,j]        = <f_i * m_i^2, f_j>
  q[i,j]          = <m_i^2, f_j^2>        (= ||f_j * m_i||^2)
  n2_i            = sum_d (f_i * m_i)^2   (= num[i,i])
  sim_masked[i,j] = num / (sqrt(n2_i) * sqrt(q))
  loss = sum_{i != j} |sim_full[i,j] - sim_masked[i,j]| / (B*(B-1))

Distribution (data-parallel over rows i): the B rows are sharded across the
8 cores (Bs = B/8 rows each).  Each core holds the full embedding table as
the moving matmul operand and computes its [Bs, B] block of the three
bilinear forms with fp8(e4m3) DoubleRow matmuls on the PE (contraction over
D), then a fused ScalarE/VectorE epilogue:
  r = 1/sqrt(n2_i * q)   (one Abs_reciprocal_sqrt activation, n2_i folded
                          in via the per-partition activation scale)
  u = sim_full - num * r
  acc[:, tile] = row-sums of |u|  (Abs activation with accum_out)
The per-core acc outputs (which include the diagonal terms) are summed on
the host; the diagonal contribution is computed exactly on the host in fp64
(O(B*D) work) and subtracted before normalizing — this avoids any per-core
control-flow divergence in the shared SPMD program.

Inputs are fed transposed (host-side layout change only) so that D lands on
the SBUF partition axis; the f32 -> bf16/fp8 casts happen on-device inside
the SWDGE DMAs.  The scalar partial sums are combined on the host (no
device collectives needed for a scalar loss).
"""

import numpy as np

import concourse.bass as bass
import concourse.tile as tile
import concourse.mybir as mybir
from concourse import bacc
from concourse.bass_utils import run_bass_kernel_spmd

F32 = mybir.dt.float32
BF16 = mybir.dt.bfloat16
FP8 = mybir.dt.float8e4
AF = mybir.ActivationFunctionType
DR = mybir.MatmulPerfMode.DoubleRow

EPS = 1e-12
N_CORES = 8


def build(B=8192, D=768, n_cores=N_CORES, NT=512, reps=1):
    """Build the SPMD Bacc program (identical on every core; all per-core
    variation is in the input data).  reps>1 wraps the body in an on-device
    loop (used only for timing experiments)."""
    Bs = B // n_cores          # rows per core
    K = D // 128               # contraction slabs
    MT = Bs // 128             # m (row) tiles per core
    JT = B // NT               # j (column) tiles
    assert D % 256 == 0 and Bs % 128 == 0 and B % NT == 0

    nc = bacc.Bacc("TRN2", target_bir_lowering=False, debug=False,
                   num_devices=n_cores)

    fT_d = nc.dram_tensor("fT", [D, B], F32, kind="ExternalInput").ap()
    fTs_d = nc.dram_tensor("fTs", [D, Bs], F32, kind="ExternalInput").ap()
    mT_d = nc.dram_tensor("mT", [D, Bs], F32, kind="ExternalInput").ap()
    acc_d = nc.dram_tensor("acc", [128, MT * JT], F32,
                           kind="ExternalOutput").ap()

    with tile.TileContext(nc) as tc:
        with (
            tc.tile_pool(name="big", bufs=1) as big,
            tc.tile_pool(name="prep", bufs=1) as prep,
            tc.tile_pool(name="f2pool", bufs=2) as f2pool,
            tc.tile_pool(name="epi", bufs=4) as epi,
            tc.tile_pool(name="junkp", bufs=2) as junkp,
            tc.tile_pool(name="psf", bufs=3, space="PSUM") as psf,
            tc.tile_pool(name="psn", bufs=2, space="PSUM") as psn,
            tc.tile_pool(name="psq", bufs=2, space="PSUM") as psq,
            tc.tile_pool(name="psn2", bufs=1, space="PSUM") as psn2p,
        ):
            fT_mm = big.tile([128, K, B], FP8)      # moving operand (fp8)
            fTs_bf = big.tile([128, K, Bs], BF16)   # f shard bf16 (prep)
            fTs_mm = big.tile([128, K, Bs], FP8)    # lhsT for sim_full
            mT_bf = prep.tile([128, K, Bs], BF16)
            m2b = prep.tile([128, K, Bs], BF16)     # m^2 bf16
            m2T = big.tile([128, K, Bs], FP8)       # lhsT for q
            aT = big.tile([128, K, Bs], BF16)       # f*m^2 bf16 (prep)
            aT_mm = big.tile([128, K, Bs], FP8)     # lhsT for num
            w2T = prep.tile([128, K, Bs], BF16)     # (f*m)^2 for n2
            ones = big.tile([128, 1], BF16)
            biasT = big.tile([128, 1], F32)
            n2_sb = big.tile([128, MT], F32)
            acc_sb = big.tile([128, MT * JT], F32)

            def body():
                # --- DMAs (SWDGE casts f32->bf16/fp8 in flight) --------
                nc.gpsimd.dma_start(
                    fTs_bf[:], fTs_d.rearrange("(k p) n -> p k n", p=128))
                nc.gpsimd.dma_start(
                    fTs_mm[:], fTs_d.rearrange("(k p) n -> p k n", p=128))
                nc.gpsimd.dma_start(
                    mT_bf[:], mT_d.rearrange("(k p) n -> p k n", p=128))
                # fT streamed j-chunk-major: early j columns of all K slabs
                # land first so the PE can start after the first chunk; the
                # first chunk is one j-panel to minimize the pipeline fill.
                bounds = [0, min(NT, B)]
                while bounds[-1] < B:
                    bounds.append(min(bounds[-1] + 2048, B))
                for jc0, jc1 in zip(bounds[:-1], bounds[1:]):
                    for kk in range(K):
                        nc.gpsimd.dma_start(
                            fT_mm[:, kk, jc0:jc1],
                            fT_d[kk * 128:(kk + 1) * 128, jc0:jc1])

                # --- prep: stationary operands -------------------------
                nc.vector.tensor_mul(m2b[:], mT_bf[:], mT_bf[:])
                nc.vector.tensor_copy(m2T[:], m2b[:])
                nc.vector.tensor_mul(aT[:], fTs_bf[:], m2b[:])
                nc.vector.tensor_copy(aT_mm[:], aT[:])
                nc.vector.tensor_mul(w2T[:], aT[:], fTs_bf[:])
                nc.vector.memset(ones[:], 1.0)
                nc.vector.memset(biasT[:], 1e-30)

                # n2_i = sum_d w2T[d, i]: matmul against a ones column
                pn2 = psn2p.tile([128, MT], F32, tag="pn2")
                for mt in range(MT):
                    for kk in range(K):
                        nc.tensor.matmul(
                            pn2[:, mt:mt + 1],
                            w2T[:, kk, mt * 128:(mt + 1) * 128],
                            ones[:],
                            start=(kk == 0), stop=(kk == K - 1))
                nc.vector.tensor_copy(n2_sb[:], pn2[:])

                # --- main loop -----------------------------------------
                for jt in range(JT):
                    j0 = jt * NT
                    f2p = f2pool.tile([128, K, NT], FP8, tag="f2p")
                    for kk in range(K):
                        if kk % 2 == 1:     # split squares across ACT/DVE
                            nc.scalar.activation(
                                f2p[:, kk, :], fT_mm[:, kk, j0:j0 + NT],
                                AF.Square)
                        else:
                            nc.vector.tensor_mul(
                                f2p[:, kk, :],
                                fT_mm[:, kk, j0:j0 + NT],
                                fT_mm[:, kk, j0:j0 + NT])
                    for mt in range(MT):
                        t_idx = jt * MT + mt
                        m0 = mt * 128
                        pf = psf.tile([128, NT], F32, tag="pf")
                        pn = psn.tile([128, NT], F32, tag="pn")
                        pq = psq.tile([128, NT], F32, tag="pq")
                        # q group first: its epilogue consumer starts
                        # earliest; pf last (freed latest, bufs=3 absorbs)
                        for kk in range(0, K, 2):
                            nc.tensor.matmul(
                                pq[:], m2T[:, kk:kk + 2, m0:m0 + 128],
                                f2p[:, kk:kk + 2, :],
                                start=(kk == 0), stop=(kk == K - 2),
                                perf_mode=DR)
                        for kk in range(0, K, 2):
                            nc.tensor.matmul(
                                pn[:], aT_mm[:, kk:kk + 2, m0:m0 + 128],
                                fT_mm[:, kk:kk + 2, j0:j0 + NT],
                                start=(kk == 0), stop=(kk == K - 2),
                                perf_mode=DR)
                        for kk in range(0, K, 2):
                            nc.tensor.matmul(
                                pf[:], fTs_mm[:, kk:kk + 2, m0:m0 + 128],
                                fT_mm[:, kk:kk + 2, j0:j0 + NT],
                                start=(kk == 0), stop=(kk == K - 2),
                                perf_mode=DR)
                        # epilogue
                        r = epi.tile([128, NT], F32, tag="r")
                        nc.scalar.activation(r[:], pq[:],
                                             AF.Abs_reciprocal_sqrt,
                                             bias=biasT[:],
                                             scale=n2_sb[:, mt:mt + 1])
                        s = epi.tile([128, NT], F32, tag="s")
                        nc.vector.tensor_mul(s[:], pn[:], r[:])
                        u = epi.tile([128, NT], F32, tag="u")
                        nc.vector.tensor_sub(u[:], pf[:], s[:])
                        junk = junkp.tile([128, NT], BF16)
                        nc.scalar.activation(
                            junk[:], u[:], AF.Abs,
                            accum_out=acc_sb[:, t_idx:t_idx + 1])

                nc.sync.dma_start(acc_d[:], acc_sb[:])

            if reps == 1:
                body()
            else:
                with tc.For_i(0, reps, 1):
                    body()

    nc.compile()
    return nc, dict(B=B, D=D, n_cores=n_cores, Bs=Bs, K=K, MT=MT, JT=JT,
                    NT=NT)


def host_inputs(full_emb, query_mask, n_cores=N_CORES):
    """Shard + transpose (layout only; all arithmetic stays on device)."""
    B, D = full_emb.shape
    Bs = B // n_cores
    fT = np.ascontiguousarray(full_emb.T)
    in_maps = []
    for c in range(n_cores):
        rows = slice(c * Bs, (c + 1) * Bs)
        in_maps.append({
            "fT": fT,
            "fTs": np.ascontiguousarray(full_emb[rows].T),
            "mT": np.ascontiguousarray(query_mask[rows].T),
        })
    return in_maps


def host_finalize(accs, full_emb, query_mask):
    """Combine per-core partial sums, subtract the diagonal, normalize."""
    B, D = full_emb.shape
    total = float(sum(a.sum(dtype=np.float64) for a in accs))
    f = full_emb.astype(np.float64)
    m = query_mask.astype(np.float64)
    num_d = ((f * m) ** 2).sum(axis=1)   # num[i,i] = n2_i = q[i,i]
    n_i = np.maximum(np.sqrt(num_d), EPS)
    sim_masked_d = num_d / (n_i * np.maximum(np.sqrt(num_d), EPS))
    sim_full_d = (f * f).sum(axis=1)
    diag = np.abs(sim_full_d - sim_masked_d).sum()
    return np.float32((total - diag) / (B * (B - 1)))


_CACHE = {}


def kernel(full_emb, query_mask):
    full_emb = np.asarray(full_emb, dtype=np.float32)
    query_mask = np.asarray(query_mask, dtype=np.float32)
    B, D = full_emb.shape
    key = (B, D)
    if key not in _CACHE:
        _CACHE[key] = build(B=B, D=D, n_cores=N_CORES)
    nc, meta = _CACHE[key]
    in_maps = host_inputs(full_emb, query_mask, N_CORES)
    res = run_bass_kernel_spmd(nc, in_maps, list(range(N_CORES)))
    accs = [res.results[c]["acc"] for c in range(N_CORES)]
    return host_finalize(accs, full_emb, query_mask)
